# revision 2
# baseline (speedup 1.0000x reference)
"""Trainium2 Bass kernel for nn_MoEDynamics (gnn_message_passing).

Reference computation (per batch sample b, rolled out `steps` times):
    h = enc(gt[:,0])                                    # [K, S]
    per step:
      w   = softmax(rout(h))                            # [K, M]
      ds  = fs(LN(h))                                   # [K, S]
      a_m = h @ w1a_m ; c_m = h @ w1b_m                 # [K, S] each
      G_m[j,s]  = sum_i relu(a_m[i,s] + c_m[j,s] + b1_m[s])
      di[j,t]   = (1/K) sum_m w[j,m] * (G_m @ w2_m)[j,t] + sum_m w[j,m] b2_m[t]
      h   = up(LN(cat[ds, di])) ; pred = dec(h) ; h = enc(pred)

Key algebraic move: sum_i commutes with the (linear) w2 matmul, so the
[K,K,S] pair tensor is reduced over i *before* w2 — a 64x FLOP cut.

Distribution: pure data-parallel over B=8 across the 8 NeuronCores (one
sample per core); no collectives needed.

On-chip layout: every activation is stored transposed, feature-on-partitions
([S or D partitions, K=64 free]); weight matrices in natural [in, out] layout
are directly the matmul lhsT. LN affine (g, bt) is folded into the following
matmul weights host-side; 1/K is folded into the softmax normalization;
exp_b2 enters as one extra matmul (against K-scaled biases) in the
delta-inter PSUM accumulation group.

The pairwise-relu core is split across two engine pipelines:
  * DVE path (experts in V_EXPERTS): per (m,i) one fused
    tensor_scalar(add, max 0) -> hid_i[s,j] tile; PE accumulates sum_i via an
    identity-lhsT PSUM chain.
  * ACT path (experts in A_EXPERTS): per (m,j) one
    activation(Relu, bias=c[:,j], accum_out=G[:,j]).
"""

import sys

sys.path.insert(0, "/opt/trn_rl_repo")

import numpy as np

import concourse.mybir as mybir
import concourse.tile as tile
from concourse import bacc
from concourse.bass_utils import run_bass_kernel_spmd

F32 = mybir.dt.float32
AF = mybir.ActivationFunctionType
ALU = mybir.AluOpType

B, T, K, D, S, M = 8, 9, 64, 8, 128, 8
N_CORES = 8
EPS = 1e-5

# expert split between the two engine pipelines (tunable)
A_EXPERTS = (0, 1)
V_EXPERTS = (2, 3, 4, 5, 6, 7)


# ---------------------------------------------------------------- wbig layout
class _Cols:
    def __init__(self):
        self.n = 0

    def take(self, w):
        s = self.n
        self.n += w
        return s


COL = {}
_c = _Cols()
for m in range(M):
    COL[f"w1a{m}"] = _c.take(S)
    COL[f"w1b{m}"] = _c.take(S)
    COL[f"w2{m}"] = _c.take(S)
for _n in ("enc_w2", "rout_w1", "fs_w1p", "fs_w2", "up_w1a", "up_w1b", "up_w2",
           "dec_w1"):
    COL[_n] = _c.take(S)
COL["rout_w2"] = _c.take(M)
COL["dec_w2"] = _c.take(D)
COL["ident"] = _c.take(S)
for _n in ("enc_b1", "enc_b2", "rout_b1", "fs_b1p", "fs_b2", "up_b1p", "up_b2",
           "dec_b1"):
    COL[_n] = _c.take(1)
for m in range(M):
    COL[f"exp_b1{m}"] = _c.take(1)
COL["ones"] = _c.take(1)
COL["zero"] = _c.take(1)
NW = _c.n

# wsmall [8, NS] column layout
SCOL = {"enc_w1": 0, "exp_b2K": 128, "i8": 256, "ones_row": 264, "sel": 392,
        "rout_b2": 392 + 8 * 128, "dec_b2": 393 + 8 * 128}
NS = 394 + 8 * 128


def _pack_weights(inp):
    """Host-side packing of all weights into wbig [128, NW] / wsmall [8, NS]."""
    f = np.float32
    exp_w1 = np.asarray(inp["exp_w1"], f)      # [M, 2S, S]
    w1a, w1b = exp_w1[:, :S, :], exp_w1[:, S:, :]
    fs_g, fs_bt = np.asarray(inp["fs_g"], f), np.asarray(inp["fs_bt"], f)
    fs_w1, fs_b1 = np.asarray(inp["fs_w1"], f), np.asarray(inp["fs_b1"], f)
    up_g, up_bt = np.asarray(inp["up_g"], f), np.asarray(inp["up_bt"], f)
    up_w1, up_b1 = np.asarray(inp["up_w1"], f), np.asarray(inp["up_b1"], f)

    fs_w1p = fs_g[:, None] * fs_w1
    fs_b1p = fs_b1 + fs_bt @ fs_w1
    up_w1p = up_g[:, None] * up_w1            # [2S, S]
    up_b1p = up_b1 + up_bt @ up_w1

    wbig = np.zeros((S, NW), f)

    def put(name, mat):
        mat = np.asarray(mat, f)
        if mat.ndim == 1:
            mat = mat[:, None]
        wbig[: mat.shape[0], COL[name]: COL[name] + mat.shape[1]] = mat

    for m in range(M):
        put(f"w1a{m}", w1a[m])
        put(f"w1b{m}", w1b[m])
        put(f"w2{m}", np.asarray(inp["exp_w2"], f)[m])
        put(f"exp_b1{m}", np.asarray(inp["exp_b1"], f)[m])
    put("enc_w2", inp["enc_w2"])
    put("rout_w1", inp["rout_w1"])
    put("fs_w1p", fs_w1p)
    put("fs_w2", inp["fs_w2"])
    put("up_w1a", up_w1p[:S])
    put("up_w1b", up_w1p[S:])
    put("up_w2", inp["up_w2"])
    put("dec_w1", inp["dec_w1"])
    put("rout_w2", inp["rout_w2"])
    put("dec_w2", inp["dec_w2"])
    put("ident", np.eye(S, dtype=f))
    put("enc_b1", inp["enc_b1"])
    put("enc_b2", inp["enc_b2"])
    put("rout_b1", inp["rout_b1"])
    put("fs_b1p", fs_b1p)
    put("fs_b2", inp["fs_b2"])
    put("up_b1p", up_b1p)
    put("up_b2", inp["up_b2"])
    put("dec_b1", inp["dec_b1"])
    put("ones", np.ones((S, 1), f))
    # "zero" column left as zeros

    wsmall = np.zeros((D, NS), f)
    wsmall[:, SCOL["enc_w1"]: SCOL["enc_w1"] + S] = np.asarray(inp["enc_w1"], f)
    wsmall[:, SCOL["exp_b2K"]: SCOL["exp_b2K"] + S] = (
        np.asarray(inp["exp_b2"], f) * np.float32(K)
    )
    wsmall[:, SCOL["i8"]: SCOL["i8"] + 8] = np.eye(8, dtype=f)
    wsmall[0:1, SCOL["ones_row"]: SCOL["ones_row"] + S] = 1.0
    for m in range(M):
        # sel_m[q, s] = (q == m): selector row matrix for the Wb broadcast
        wsmall[m, SCOL["sel"] + m * S: SCOL["sel"] + (m + 1) * S] = 1.0
    wsmall[:, SCOL["rout_b2"]: SCOL["rout_b2"] + 1] = (
        np.asarray(inp["rout_b2"], f)[:, None]
    )
    wsmall[:, SCOL["dec_b2"]: SCOL["dec_b2"] + 1] = (
        np.asarray(inp["dec_b2"], f)[:, None]
    )
    return wbig, wsmall


# ------------------------------------------------------------------ builder
def build_program(steps):
    nc = bacc.Bacc("TRN2", target_bir_lowering=False, debug=False,
                   num_devices=N_CORES)
    wbig_d = nc.dram_tensor("wbig", [S, NW], F32, kind="ExternalInput")
    wsmall_d = nc.dram_tensor("wsmall", [D, NS], F32, kind="ExternalInput")
    x0_d = nc.dram_tensor("x0t", [D, K], F32, kind="ExternalInput")
    pred_d = nc.dram_tensor("pred", [steps * K, D], F32, kind="ExternalOutput")

    with tile.TileContext(nc) as tc:
        _build_body(nc, tc, wbig_d, wsmall_d, x0_d, pred_d, steps)
    nc.compile()
    return nc


def _build_body(nc, tc, wbig_d, wsmall_d, x0_d, pred_d, steps):
    from contextlib import ExitStack

    ctx = ExitStack()
    with ctx:
        pconst = ctx.enter_context(tc.tile_pool(name="const", bufs=1))
        pact = ctx.enter_context(tc.tile_pool(name="act", bufs=4))
        phid = ctx.enter_context(tc.tile_pool(name="hid", bufs=12))
        pgs = ctx.enter_context(tc.tile_pool(name="gsb", bufs=2))
        psml = ctx.enter_context(tc.tile_pool(name="sml", bufs=4))
        # PSUM pools — 8 banks total:
        #   pmlp tag "mlp" (2) + pgps (2) + pdel (1) + pwb (1) + pstat (1)
        #   + pbc (1)
        pmlp = ctx.enter_context(tc.tile_pool(name="mlp", bufs=2, space="PSUM"))
        pgps = ctx.enter_context(tc.tile_pool(name="gps", bufs=2, space="PSUM"))
        pdel = ctx.enter_context(tc.tile_pool(name="del", bufs=1, space="PSUM"))
        pwb = ctx.enter_context(tc.tile_pool(name="wb", bufs=1, space="PSUM"))
        pstat = ctx.enter_context(tc.tile_pool(name="stat", bufs=1, space="PSUM"))
        pbc = ctx.enter_context(tc.tile_pool(name="bc", bufs=1, space="PSUM"))

        wb = pconst.tile([S, NW], F32)
        ws = pconst.tile([D, NS], F32)
        x0 = pconst.tile([D, K], F32)
        pred_all = pconst.tile([D, steps * K], F32)
        nc.sync.dma_start(wb[:], wbig_d[:])
        nc.sync.dma_start(ws[:], wsmall_d[:])
        nc.sync.dma_start(x0[:], x0_d[:])

        def W(name, width=S):
            c = COL[name]
            return wb[:, c: c + width]

        def bias(name):
            c = COL[name]
            return wb[:, c: c + 1]

        ident = W("ident")
        onescol = bias("ones")                      # [128, 1] of 1.0
        zerocol = bias("zero")                      # [128, 1] of 0.0
        ones_row = ws[0:1, SCOL["ones_row"]: SCOL["ones_row"] + S]  # [1, 128]
        i8 = ws[:, SCOL["i8"]: SCOL["i8"] + 8]
        enc_w1 = ws[:, SCOL["enc_w1"]: SCOL["enc_w1"] + S]
        exp_b2K = ws[:, SCOL["exp_b2K"]: SCOL["exp_b2K"] + S]
        rout_b2 = ws[:, SCOL["rout_b2"]: SCOL["rout_b2"] + 1]
        dec_b2 = ws[:, SCOL["dec_b2"]: SCOL["dec_b2"] + 1]

        def sel(m):
            return ws[:, SCOL["sel"] + m * S: SCOL["sel"] + (m + 1) * S]

        def mlp_8toS(x_in, b1ap, b2ap):
            """encoder: [8,K] -> relu(enc_w1.T x + b1) -> enc_w2.T . + b2."""
            p1 = pmlp.tile([S, K], F32, tag="mlp")
            nc.tensor.matmul(p1[:], enc_w1, x_in, start=True, stop=True)
            mid = pact.tile([S, K], F32, tag="mid")
            nc.scalar.activation(mid[:], p1[:], AF.Relu, bias=b1ap)
            p2 = pmlp.tile([S, K], F32, tag="mlp")
            nc.tensor.matmul(p2[:], W("enc_w2"), mid[:], start=True, stop=True)
            out = pact.tile([S, K], F32, tag="h")
            nc.scalar.activation(out[:], p2[:], AF.Identity, bias=b2ap)
            return out

        # ---- h0 = enc(x0) ----
        hT = mlp_8toS(x0, bias("enc_b1"), bias("enc_b2"))

        for s in range(steps):
            # ============ router: wn = softmax(rout(h)) / K ============
            r1 = pmlp.tile([S, K], F32, tag="mlp")
            nc.tensor.matmul(r1[:], W("rout_w1"), hT[:], start=True, stop=True)
            r1s = pact.tile([S, K], F32, tag="mid")
            nc.scalar.activation(r1s[:], r1[:], AF.Relu, bias=bias("rout_b1"))
            lg = pmlp.tile([M, K], F32, tag="mlp")
            nc.tensor.matmul(lg[:], W("rout_w2", M), r1s[:], start=True, stop=True)
            exps = pact.tile([M, K], F32, tag="exps")
            nc.scalar.activation(exps[:], lg[:], AF.Exp, bias=rout_b2)

            stat = pstat.tile([M, 448], F32, tag="stat")
            nc.tensor.matmul(stat[0:1, 128:192], onescol[0:8, :], exps[:],
                             start=True, stop=True)
            rden = psml.tile([1, K], F32, tag="rden")
            nc.vector.reciprocal(rden[:], stat[0:1, 128:192])
            nc.vector.tensor_scalar(rden[:], rden[:], 1.0 / K, None, op0=ALU.mult)
            # rden broadcast to 8 rows, then wn = exps * rden (includes 1/K)
            nc.tensor.matmul(stat[0:8, 384:448], ones_row[0:1, 0:8], rden[:],
                             start=True, stop=True)
            wn = pact.tile([M, K], F32, tag="wn")
            nc.vector.tensor_tensor(wn[:], exps[:], stat[0:8, 384:448],
                                    op=ALU.mult)
            # broadcast weight rows to [S, K] tiles (one PSUM bank, col slices)
            wbps = pwb.tile([S, M * K], F32, tag="wb")
            for m in range(M):
                nc.tensor.matmul(wbps[:, m * K: (m + 1) * K], sel(m), wn[:],
                                 start=True, stop=True)
            wbsb = pgs.tile([S, M * K], F32, tag="wbsb")
            for m in range(M):
                nc.scalar.copy(wbsb[:, m * K: (m + 1) * K],
                               wbps[:, m * K: (m + 1) * K])

            # ============ fs-LN + fs MLP -> delta_self ============
            nc.tensor.matmul(stat[0:1, 0:64], onescol, hT[:], start=True,
                             stop=True)
            hsq = pact.tile([S, K], F32, tag="hsq")
            nc.vector.tensor_tensor(hsq[:], hT[:], hT[:], op=ALU.mult)
            nc.tensor.matmul(stat[0:1, 64:128], onescol, hsq[:], start=True,
                             stop=True)

            def ln_stats(mu_ap, sq_ap, inv_n, tag):
                mean = psml.tile([1, K], F32, tag=tag + "mean")
                nc.vector.tensor_scalar(mean[:], mu_ap, inv_n, None, op0=ALU.mult)
                var = psml.tile([1, K], F32, tag=tag + "var")
                nc.vector.tensor_scalar(var[:], sq_ap, inv_n, None, op0=ALU.mult)
                msq = psml.tile([1, K], F32, tag=tag + "msq")
                nc.vector.tensor_tensor(msq[:], mean[:], mean[:], op=ALU.mult)
                nc.vector.tensor_tensor(var[:], var[:], msq[:], op=ALU.subtract)
                nc.vector.tensor_scalar(var[:], var[:], EPS, None, op0=ALU.add)
                lnv = psml.tile([1, K], F32, tag=tag + "lnv")
                nc.scalar.activation(lnv[:], var[:], AF.Ln, bias=zerocol[0:1, :])
                rstd = psml.tile([1, K], F32, tag=tag + "rstd")
                nc.scalar.activation(rstd[:], lnv[:], AF.Exp, scale=-0.5,
                                     bias=zerocol[0:1, :])
                mrs = psml.tile([1, K], F32, tag=tag + "mrs")
                nc.vector.tensor_tensor(mrs[:], mean[:], rstd[:], op=ALU.mult)
                return rstd, mrs

            rstd, mrs = ln_stats(stat[0:1, 0:64], stat[0:1, 64:128], 1.0 / S, "f")
            bc = pbc.tile([S, 256], F32, tag="bc")
            nc.tensor.matmul(bc[:, 0:64], ones_row, rstd[:], start=True, stop=True)
            nc.tensor.matmul(bc[:, 64:128], ones_row, mrs[:], start=True, stop=True)
            hn = pact.tile([S, K], F32, tag="hn")
            nc.vector.tensor_tensor(hn[:], hT[:], bc[:, 0:64], op=ALU.mult)
            nc.vector.tensor_tensor(hn[:], hn[:], bc[:, 64:128], op=ALU.subtract)

            f1 = pmlp.tile([S, K], F32, tag="mlp")
            nc.tensor.matmul(f1[:], W("fs_w1p"), hn[:], start=True, stop=True)
            f1s = pact.tile([S, K], F32, tag="mid")
            nc.scalar.activation(f1s[:], f1[:], AF.Relu, bias=bias("fs_b1p"))
            f2 = pmlp.tile([S, K], F32, tag="mlp")
            nc.tensor.matmul(f2[:], W("fs_w2"), f1s[:], start=True, stop=True)
            dsT = pact.tile([S, K], F32, tag="ds")
            nc.scalar.activation(dsT[:], f2[:], AF.Identity, bias=bias("fs_b2"))

            # ============ experts ============
            aT = {}
            cT = {}
            for m in A_EXPERTS + V_EXPERTS:
                pa = pmlp.tile([S, K], F32, tag="mlp")
                nc.tensor.matmul(pa[:], W(f"w1a{m}"), hT[:], start=True, stop=True)
                at = pact.tile([S, K], F32, tag=f"a{m}")
                nc.scalar.activation(at[:], pa[:], AF.Identity,
                                     bias=bias(f"exp_b1{m}"))
                aT[m] = at
                pc = pmlp.tile([S, K], F32, tag="mlp")
                nc.tensor.matmul(pc[:], W(f"w1b{m}"), hT[:], start=True, stop=True)
                ct = pact.tile([S, K], F32, tag=f"c{m}")
                nc.scalar.copy(ct[:], pc[:])
                cT[m] = ct

            dps = pdel.tile([S, K], F32, tag="delta")
            # exp_b2 term opens the accumulation group
            nc.tensor.matmul(dps[:], exp_b2K, wn[:], start=True, stop=False)

            gprime = {}
            # ACT path: per (m, j) relu + accumulate
            for m in A_EXPERTS:
                gsb = pgs.tile([S, K], F32, tag=f"gact{m}")
                scratch = pgs.tile([S, K], F32, tag="scr")
                for j in range(K):
                    nc.scalar.activation(scratch[:], aT[m][:], AF.Relu,
                                         bias=cT[m][:, j: j + 1],
                                         accum_out=gsb[:, j: j + 1])
                gp = pgs.tile([S, K], F32, tag=f"gp{m}")
                nc.vector.tensor_tensor(gp[:], gsb[:],
                                        wbsb[:, m * K: (m + 1) * K], op=ALU.mult)
                gprime[m] = gp

            # DVE path: per (m, i) fused tensor_scalar + PE identity chain
            for m in V_EXPERTS:
                gps = pgps.tile([S, K], F32, tag="gchain")
                for i in range(K):
                    hid = phid.tile([S, K], F32, tag="hid")
                    nc.vector.tensor_scalar(hid[:], cT[m][:], aT[m][:, i: i + 1],
                                            0.0, op0=ALU.add, op1=ALU.max)
                    nc.tensor.matmul(gps[:], ident, hid[:], start=(i == 0),
                                     stop=(i == K - 1))
                gp = pgs.tile([S, K], F32, tag=f"gp{m}")
                nc.vector.tensor_tensor(gp[:], gps[:],
                                        wbsb[:, m * K: (m + 1) * K], op=ALU.mult)
                gprime[m] = gp

            for n, m in enumerate(A_EXPERTS + V_EXPERTS):
                nc.tensor.matmul(dps[:], W(f"w2{m}"), gprime[m][:], start=False,
                                 stop=(n == M - 1))

            diT = pact.tile([S, K], F32, tag="di")
            nc.vector.tensor_copy(diT[:], dps[:])

            # ============ up-LN over cat[ds, di] + up MLP -> h_next ============
            nc.tensor.matmul(stat[0:1, 192:256], onescol, dsT[:], start=True,
                             stop=False)
            nc.tensor.matmul(stat[0:1, 192:256], onescol, diT[:], start=False,
                             stop=True)
            dsq = pact.tile([S, K], F32, tag="dsq")
            nc.vector.tensor_tensor(dsq[:], dsT[:], dsT[:], op=ALU.mult)
            disq = pact.tile([S, K], F32, tag="disq")
            nc.vector.tensor_tensor(disq[:], diT[:], diT[:], op=ALU.mult)
            nc.tensor.matmul(stat[0:1, 256:320], onescol, dsq[:], start=True,
                             stop=False)
            nc.tensor.matmul(stat[0:1, 256:320], onescol, disq[:], start=False,
                             stop=True)
            rstd2, mrs2 = ln_stats(stat[0:1, 192:256], stat[0:1, 256:320],
                                   1.0 / (2 * S), "u")
            nc.tensor.matmul(bc[:, 128:192], ones_row, rstd2[:], start=True,
                             stop=True)
            nc.tensor.matmul(bc[:, 192:256], ones_row, mrs2[:], start=True,
                             stop=True)
            dsn = pact.tile([S, K], F32, tag="dsn")
            nc.vector.tensor_tensor(dsn[:], dsT[:], bc[:, 128:192], op=ALU.mult)
            nc.vector.tensor_tensor(dsn[:], dsn[:], bc[:, 192:256],
                                    op=ALU.subtract)
            din = pact.tile([S, K], F32, tag="din")
            nc.vector.tensor_tensor(din[:], diT[:], bc[:, 128:192], op=ALU.mult)
            nc.vector.tensor_tensor(din[:], din[:], bc[:, 192:256],
                                    op=ALU.subtract)

            u1 = pmlp.tile([S, K], F32, tag="mlp")
            nc.tensor.matmul(u1[:], W("up_w1a"), dsn[:], start=True, stop=False)
            nc.tensor.matmul(u1[:], W("up_w1b"), din[:], start=False, stop=True)
            u1s = pact.tile([S, K], F32, tag="mid")
            nc.scalar.activation(u1s[:], u1[:], AF.Relu, bias=bias("up_b1p"))
            u2 = pmlp.tile([S, K], F32, tag="mlp")
            nc.tensor.matmul(u2[:], W("up_w2"), u1s[:], start=True, stop=True)
            hT = pact.tile([S, K], F32, tag="h")
            nc.scalar.activation(hT[:], u2[:], AF.Identity, bias=bias("up_b2"))

            # ============ dec -> pred ============
            d1 = pmlp.tile([S, K], F32, tag="mlp")
            nc.tensor.matmul(d1[:], W("dec_w1"), hT[:], start=True, stop=True)
            d1s = pact.tile([S, K], F32, tag="mid")
            nc.scalar.activation(d1s[:], d1[:], AF.Relu, bias=bias("dec_b1"))
            d2 = pmlp.tile([M, K], F32, tag="mlp")
            nc.tensor.matmul(d2[:], W("dec_w2", D), d1s[:], start=True, stop=True)
            predT = pred_all[:, s * K: (s + 1) * K]
            nc.scalar.activation(predT, d2[:], AF.Identity, bias=dec_b2)

            # pred -> HBM ([K, D] rows)
            trp = pmlp.tile([K, D], F32, tag="mlp")
            nc.tensor.transpose(trp[:], predT, i8)
            po = pact.tile([K, D], F32, tag="po")
            nc.vector.tensor_copy(po[:], trp[:])
            nc.sync.dma_start(pred_d[s * K: (s + 1) * K, :], po[:])

            # ============ re-encode (skip after the last step) ============
            if s < steps - 1:
                hT = mlp_8toS(predT, bias("enc_b1"), bias("enc_b2"))


# ------------------------------------------------------------------ runner
_CACHE = {}


def _get_program(steps):
    if steps not in _CACHE:
        _CACHE[steps] = build_program(steps)
    return _CACHE[steps]


def run_on_hw(inputs, steps, trace=False, tmpdir=None):
    nc = _get_program(steps)
    wbig, wsmall = _pack_weights(inputs)
    gt = np.asarray(inputs["gt_states"], np.float32)
    in_maps = []
    for b in range(N_CORES):
        in_maps.append({
            "wbig": wbig,
            "wsmall": wsmall,
            "x0t": np.ascontiguousarray(gt[b, 0].T),   # [D, K]
        })
    res = run_bass_kernel_spmd(nc, in_maps, list(range(N_CORES)), trace=trace,
                               tmpdir=tmpdir)
    preds = np.stack(
        [res.results[b]["pred"].reshape(steps, K, D) for b in range(N_CORES)],
        axis=0,
    )
    return preds, res


def kernel(**inputs):
    gt = np.asarray(inputs["gt_states"], np.float32)
    steps = min(T - 1, int(np.asarray(inputs["rollout_steps"])))
    target = gt[:, 1: steps + 1]
    if steps <= 0:
        return np.zeros((B, 0, K, D), np.float32), target
    preds, _ = run_on_hw(inputs, steps)
    return preds, target


# revision 3
# speedup vs baseline: 1.0845x; 1.0845x over previous
"""Trainium2 Bass kernel for nn_MoEDynamics (gnn_message_passing).

Reference computation (per batch sample b, rolled out `steps` times):
    h = enc(gt[:,0])                                    # [K, S]
    per step:
      w   = softmax(rout(h))                            # [K, M]
      ds  = fs(LN(h))                                   # [K, S]
      a_m = h @ w1a_m ; c_m = h @ w1b_m                 # [K, S] each
      G_m[j,s]  = sum_i relu(a_m[i,s] + c_m[j,s] + b1_m[s])
      di[j,t]   = (1/K) sum_m w[j,m] * (G_m @ w2_m)[j,t] + sum_m w[j,m] b2_m[t]
      h   = up(LN(cat[ds, di])) ; pred = dec(h) ; h = enc(pred)

Key algebraic move: sum_i commutes with the (linear) w2 matmul, so the
[K,K,S] pair tensor is reduced over i *before* w2 — a 64x FLOP cut.

Distribution: pure data-parallel over B=8 across the 8 NeuronCores (one
sample per core); no collectives needed.

On-chip layout: every activation is stored transposed, feature-on-partitions
([S or D partitions, K=64 free]); weight matrices in natural [in, out] layout
are directly the matmul lhsT. LN affine (g, bt) is folded into the following
matmul weights host-side; 1/K is folded into the softmax normalization;
exp_b2 enters as one extra matmul (against K-scaled biases) in the
delta-inter PSUM accumulation group.

The pairwise-relu core is split across two engine pipelines:
  * DVE path (experts in V_EXPERTS): per (m,i) one fused
    tensor_scalar(add, max 0) -> hid_i[s,j] bf16 tile; PE accumulates sum_i
    via a bf16 identity-lhsT PSUM chain (bf16 matmuls run 4x faster than
    fp32 on the PE).
  * ACT path (experts in A_EXPERTS): per (m,j) one
    activation(Relu, bias=c[:,j], accum_out=G[:,j]).

Program order is engine-FIFO-aware: per-engine queues are emitted in data-
readiness order (engines execute strictly in order, so a stalled head blocks
the queue). All ACT activations use functions from the single
natural_log_exp_and_others table set (rstd = exp(-0.5*ln(var+eps))), and the
table chooser is pinned to that set to avoid ~2.7us table reloads.
"""

import sys

sys.path.insert(0, "/opt/trn_rl_repo")

import ml_dtypes
import numpy as np

import concourse.mybir as mybir
import concourse.tile as tile
from concourse import bacc
from concourse.bass_utils import run_bass_kernel_spmd

F32 = mybir.dt.float32
BF16 = mybir.dt.bfloat16
AF = mybir.ActivationFunctionType
ALU = mybir.AluOpType

B, T, K, D, S, M = 8, 9, 64, 8, 128, 8
N_CORES = 8
EPS = 1e-5

# expert split between the two engine pipelines (tunable)
A_EXPERTS = (0, 1)
V_EXPERTS = (2, 3, 4, 5, 6, 7)

_ACT_SET = "natural_log_exp_and_others"


def _pin_act_tables():
    """Make the ACT-table chooser see only one (sufficient) function set so
    the kernel never reloads activation tables mid-flight."""
    import concourse.bacc as _bacc
    import concourse.hw_specs as _hw

    if getattr(_bacc, "_act_tables_pinned", False):
        return
    orig = _hw.get_activation_tables

    def pinned(arch):
        t = orig(arch)
        return {k: (v if k == _ACT_SET else set()) for k, v in t.items()}

    _bacc.get_activation_tables = pinned
    _bacc._act_tables_pinned = True


# ---------------------------------------------------------------- wbig layout
class _Cols:
    def __init__(self):
        self.n = 0

    def take(self, w):
        s = self.n
        self.n += w
        return s


COL = {}
_c = _Cols()
for m in range(M):
    COL[f"w1a{m}"] = _c.take(S)
    COL[f"w1b{m}"] = _c.take(S)
    COL[f"w2{m}"] = _c.take(S)
for _n in ("enc_w2", "rout_w1", "fs_w1p", "fs_w2", "up_w1a", "up_w1b", "up_w2",
           "dec_w1"):
    COL[_n] = _c.take(S)
COL["rout_w2"] = _c.take(M)
COL["dec_w2"] = _c.take(D)
for _n in ("enc_b1", "enc_b2", "rout_b1", "fs_b1p", "fs_b2", "up_b1p", "up_b2",
           "dec_b1"):
    COL[_n] = _c.take(1)
for m in range(M):
    COL[f"exp_b1{m}"] = _c.take(1)
COL["ones"] = _c.take(1)
COL["zero"] = _c.take(1)
NW = _c.n

# wsmall [8, NS] column layout
SCOL = {"enc_w1": 0, "exp_b2K": 128, "i8": 256, "ones_row": 264, "sel": 392,
        "rout_b2": 392 + 8 * 128, "dec_b2": 393 + 8 * 128}
NS = 394 + 8 * 128


def _pack_weights(inp):
    """Host-side packing into wbig [128, NW] f32 / wsmall [8, NS] f32 /
    id16 [128, 128] bf16."""
    f = np.float32
    exp_w1 = np.asarray(inp["exp_w1"], f)      # [M, 2S, S]
    w1a, w1b = exp_w1[:, :S, :], exp_w1[:, S:, :]
    fs_g, fs_bt = np.asarray(inp["fs_g"], f), np.asarray(inp["fs_bt"], f)
    fs_w1, fs_b1 = np.asarray(inp["fs_w1"], f), np.asarray(inp["fs_b1"], f)
    up_g, up_bt = np.asarray(inp["up_g"], f), np.asarray(inp["up_bt"], f)
    up_w1, up_b1 = np.asarray(inp["up_w1"], f), np.asarray(inp["up_b1"], f)

    fs_w1p = fs_g[:, None] * fs_w1
    fs_b1p = fs_b1 + fs_bt @ fs_w1
    up_w1p = up_g[:, None] * up_w1            # [2S, S]
    up_b1p = up_b1 + up_bt @ up_w1

    wbig = np.zeros((S, NW), f)

    def put(name, mat):
        mat = np.asarray(mat, f)
        if mat.ndim == 1:
            mat = mat[:, None]
        wbig[: mat.shape[0], COL[name]: COL[name] + mat.shape[1]] = mat

    for m in range(M):
        put(f"w1a{m}", w1a[m])
        put(f"w1b{m}", w1b[m])
        put(f"w2{m}", np.asarray(inp["exp_w2"], f)[m])
        put(f"exp_b1{m}", np.asarray(inp["exp_b1"], f)[m])
    put("enc_w2", inp["enc_w2"])
    put("rout_w1", inp["rout_w1"])
    put("fs_w1p", fs_w1p)
    put("fs_w2", inp["fs_w2"])
    put("up_w1a", up_w1p[:S])
    put("up_w1b", up_w1p[S:])
    put("up_w2", inp["up_w2"])
    put("dec_w1", inp["dec_w1"])
    put("rout_w2", inp["rout_w2"])
    put("dec_w2", inp["dec_w2"])
    put("enc_b1", inp["enc_b1"])
    put("enc_b2", inp["enc_b2"])
    put("rout_b1", inp["rout_b1"])
    put("fs_b1p", fs_b1p)
    put("fs_b2", inp["fs_b2"])
    put("up_b1p", up_b1p)
    put("up_b2", inp["up_b2"])
    put("dec_b1", inp["dec_b1"])
    put("ones", np.ones((S, 1), f))
    # "zero" column left as zeros

    wsmall = np.zeros((D, NS), f)
    wsmall[:, SCOL["enc_w1"]: SCOL["enc_w1"] + S] = np.asarray(inp["enc_w1"], f)
    wsmall[:, SCOL["exp_b2K"]: SCOL["exp_b2K"] + S] = (
        np.asarray(inp["exp_b2"], f) * np.float32(K)
    )
    wsmall[:, SCOL["i8"]: SCOL["i8"] + 8] = np.eye(8, dtype=f)
    wsmall[0:1, SCOL["ones_row"]: SCOL["ones_row"] + S] = 1.0
    for m in range(M):
        # sel_m[q, s] = (q == m): selector matrix for the Wb row-broadcast
        wsmall[m, SCOL["sel"] + m * S: SCOL["sel"] + (m + 1) * S] = 1.0
    wsmall[:, SCOL["rout_b2"]: SCOL["rout_b2"] + 1] = (
        np.asarray(inp["rout_b2"], f)[:, None]
    )
    wsmall[:, SCOL["dec_b2"]: SCOL["dec_b2"] + 1] = (
        np.asarray(inp["dec_b2"], f)[:, None]
    )
    id16 = np.eye(S, dtype=ml_dtypes.bfloat16)
    return wbig, wsmall, id16


# ------------------------------------------------------------------ builder
def build_program(steps):
    _pin_act_tables()
    nc = bacc.Bacc("TRN2", target_bir_lowering=False, debug=False,
                   num_devices=N_CORES)
    wbig_d = nc.dram_tensor("wbig", [S, NW], F32, kind="ExternalInput")
    wsmall_d = nc.dram_tensor("wsmall", [D, NS], F32, kind="ExternalInput")
    id16_d = nc.dram_tensor("id16", [S, S], BF16, kind="ExternalInput")
    x0_d = nc.dram_tensor("x0t", [D, K], F32, kind="ExternalInput")
    pred_d = nc.dram_tensor("pred", [steps * K, D], F32, kind="ExternalOutput")

    with tile.TileContext(nc) as tc:
        _build_body(nc, tc, wbig_d, wsmall_d, id16_d, x0_d, pred_d, steps)
    nc.compile()
    return nc


def _build_body(nc, tc, wbig_d, wsmall_d, id16_d, x0_d, pred_d, steps):
    from contextlib import ExitStack

    ctx = ExitStack()
    with ctx:
        pconst = ctx.enter_context(tc.tile_pool(name="const", bufs=1))
        pact = ctx.enter_context(tc.tile_pool(name="act", bufs=4))
        phid = ctx.enter_context(tc.tile_pool(name="hid", bufs=12))
        pgs = ctx.enter_context(tc.tile_pool(name="gsb", bufs=2))
        psml = ctx.enter_context(tc.tile_pool(name="sml", bufs=4))
        # PSUM pools — 8 banks total: pmlp(2) + pgps(2) + pdel(1) + pwb(1)
        # + pstat(1) + pbc(1)
        pmlp = ctx.enter_context(tc.tile_pool(name="mlp", bufs=2, space="PSUM"))
        pgps = ctx.enter_context(tc.tile_pool(name="gps", bufs=2, space="PSUM"))
        pdel = ctx.enter_context(tc.tile_pool(name="del", bufs=1, space="PSUM"))
        pwb = ctx.enter_context(tc.tile_pool(name="wb", bufs=1, space="PSUM"))
        pstat = ctx.enter_context(tc.tile_pool(name="stat", bufs=1, space="PSUM"))
        pbc = ctx.enter_context(tc.tile_pool(name="bc", bufs=1, space="PSUM"))

        wb = pconst.tile([S, NW], F32)
        ws = pconst.tile([D, NS], F32)
        id16 = pconst.tile([S, S], BF16)
        x0 = pconst.tile([D, K], F32)
        pred_all = pconst.tile([D, steps * K], F32)
        nc.sync.dma_start(wb[:], wbig_d[:])
        nc.sync.dma_start(ws[:], wsmall_d[:])
        nc.sync.dma_start(id16[:], id16_d[:])
        nc.sync.dma_start(x0[:], x0_d[:])

        def W(name, width=S):
            c = COL[name]
            return wb[:, c: c + width]

        def bias(name):
            c = COL[name]
            return wb[:, c: c + 1]

        onescol = bias("ones")                      # [128, 1] of 1.0
        zerocol = bias("zero")                      # [128, 1] of 0.0
        ones_row = ws[0:1, SCOL["ones_row"]: SCOL["ones_row"] + S]  # [1, 128]
        i8 = ws[:, SCOL["i8"]: SCOL["i8"] + 8]
        enc_w1 = ws[:, SCOL["enc_w1"]: SCOL["enc_w1"] + S]
        exp_b2K = ws[:, SCOL["exp_b2K"]: SCOL["exp_b2K"] + S]
        rout_b2 = ws[:, SCOL["rout_b2"]: SCOL["rout_b2"] + 1]
        dec_b2 = ws[:, SCOL["dec_b2"]: SCOL["dec_b2"] + 1]

        def sel(m):
            return ws[:, SCOL["sel"] + m * S: SCOL["sel"] + (m + 1) * S]

        def mlp_8toS(x_in, b1ap, b2ap):
            """encoder: [8,K] -> relu(enc_w1.T x + b1) -> enc_w2.T . + b2."""
            p1 = pmlp.tile([S, K], F32, tag="mlp")
            nc.tensor.matmul(p1[:], enc_w1, x_in, start=True, stop=True)
            mid = pact.tile([S, K], F32, tag="mid")
            nc.scalar.activation(mid[:], p1[:], AF.Relu, bias=b1ap)
            p2 = pmlp.tile([S, K], F32, tag="mlp")
            nc.tensor.matmul(p2[:], W("enc_w2"), mid[:], start=True, stop=True)
            out = pact.tile([S, K], F32, tag="h")
            nc.scalar.activation(out[:], p2[:], AF.Identity, bias=b2ap)
            return out

        def ln_stats(mu_ap, sq_ap, inv_n, tag):
            """[1,K] LN stats -> (rstd, mean*rstd); exp/ln only (one ACT set)."""
            mean = psml.tile([1, K], F32, tag=tag + "mean")
            nc.vector.tensor_scalar(mean[:], mu_ap, inv_n, None, op0=ALU.mult)
            var = psml.tile([1, K], F32, tag=tag + "var")
            nc.vector.tensor_scalar(var[:], sq_ap, inv_n, None, op0=ALU.mult)
            msq = psml.tile([1, K], F32, tag=tag + "msq")
            nc.vector.tensor_tensor(msq[:], mean[:], mean[:], op=ALU.mult)
            nc.vector.tensor_tensor(var[:], var[:], msq[:], op=ALU.subtract)
            nc.vector.tensor_scalar(var[:], var[:], EPS, None, op0=ALU.add)
            lnv = psml.tile([1, K], F32, tag=tag + "lnv")
            nc.scalar.activation(lnv[:], var[:], AF.Ln, bias=zerocol[0:1, :])
            rstd = psml.tile([1, K], F32, tag=tag + "rstd")
            nc.scalar.activation(rstd[:], lnv[:], AF.Exp, scale=-0.5,
                                 bias=zerocol[0:1, :])
            mrs = psml.tile([1, K], F32, tag=tag + "mrs")
            nc.vector.tensor_tensor(mrs[:], mean[:], rstd[:], op=ALU.mult)
            return rstd, mrs

        # ---- h0 = enc(x0) ----
        hT = mlp_8toS(x0, bias("enc_b1"), bias("enc_b2"))

        for s in range(steps):
            # ---- expert inputs a_m (+b1), c_m — V-experts first so the DVE
            # pipeline unblocks as early as possible.
            aT = {}
            cT = {}
            for m in V_EXPERTS + A_EXPERTS:
                pa = pmlp.tile([S, K], F32, tag="mlp")
                nc.tensor.matmul(pa[:], W(f"w1a{m}"), hT[:], start=True,
                                 stop=True)
                at = pact.tile([S, K], F32, tag=f"a{m}")
                nc.scalar.activation(at[:], pa[:], AF.Identity,
                                     bias=bias(f"exp_b1{m}"))
                aT[m] = at
                pc = pmlp.tile([S, K], F32, tag="mlp")
                nc.tensor.matmul(pc[:], W(f"w1b{m}"), hT[:], start=True,
                                 stop=True)
                # V-expert c tiles in bf16: feeds the 4x-mode tensor_scalar
                ct = pact.tile([S, K], BF16 if m in V_EXPERTS else F32,
                               tag=f"c{m}")
                nc.scalar.copy(ct[:], pc[:])
                cT[m] = ct

            # ---- router MLP (PE/ACT interleave with the drains above)
            r1 = pmlp.tile([S, K], F32, tag="mlp")
            nc.tensor.matmul(r1[:], W("rout_w1"), hT[:], start=True, stop=True)
            r1s = pact.tile([S, K], F32, tag="mid")
            nc.scalar.activation(r1s[:], r1[:], AF.Relu, bias=bias("rout_b1"))
            lg = pmlp.tile([M, K], F32, tag="mlp")
            nc.tensor.matmul(lg[:], W("rout_w2", M), r1s[:], start=True,
                             stop=True)
            exps = pact.tile([M, K], F32, tag="exps")
            nc.scalar.activation(exps[:], lg[:], AF.Exp, bias=rout_b2)

            # ---- first V-expert TS block (DVE queue head: only needs a/c)
            gchains = {}

            def v_chain(m):
                gps = pgps.tile([S, K], F32, tag="gchain")
                for i in range(K):
                    hid = phid.tile([S, K], BF16, tag="hid")
                    nc.vector.tensor_scalar(hid[:], cT[m][:],
                                            aT[m][:, i: i + 1], 0.0,
                                            op0=ALU.add, op1=ALU.max)
                    nc.tensor.matmul(gps[:], id16[:], hid[:], start=(i == 0),
                                     stop=(i == K - 1))
                gchains[m] = gps

            v_chain(V_EXPERTS[0])

            # ---- router weight normalization (DVE smalls; by now exps ready)
            stat = pstat.tile([M, 448], F32, tag="stat")
            nc.tensor.matmul(stat[0:1, 128:192], onescol[0:8, :], exps[:],
                             start=True, stop=True)
            rden = psml.tile([1, K], F32, tag="rden")
            nc.vector.reciprocal(rden[:], stat[0:1, 128:192])
            nc.vector.tensor_scalar(rden[:], rden[:], 1.0 / K, None,
                                    op0=ALU.mult)
            nc.tensor.matmul(stat[0:8, 384:448], ones_row[0:1, 0:8], rden[:],
                             start=True, stop=True)
            wn = pact.tile([M, K], F32, tag="wn")
            nc.vector.tensor_tensor(wn[:], exps[:], stat[0:8, 384:448],
                                    op=ALU.mult)
            # row-broadcast weights into one PSUM bank; V-experts also get an
            # SBUF copy (their G sits in PSUM, TT needs one SBUF operand)
            wbps = pwb.tile([S, M * K], F32, tag="wb")
            for m in range(M):
                nc.tensor.matmul(wbps[:, m * K: (m + 1) * K], sel(m), wn[:],
                                 start=True, stop=True)
            wbsb = pgs.tile([S, M * K], F32, tag="wbsb")
            for m in V_EXPERTS:
                nc.vector.tensor_copy(wbsb[:, m * K: (m + 1) * K],
                                      wbps[:, m * K: (m + 1) * K])

            # ---- fs-LN stats (DVE smalls) + fs MLP
            nc.tensor.matmul(stat[0:1, 0:64], onescol, hT[:], start=True,
                             stop=True)
            hsq = pact.tile([S, K], F32, tag="hsq")
            nc.vector.tensor_tensor(hsq[:], hT[:], hT[:], op=ALU.mult)
            nc.tensor.matmul(stat[0:1, 64:128], onescol, hsq[:], start=True,
                             stop=True)
            rstd, mrs = ln_stats(stat[0:1, 0:64], stat[0:1, 64:128], 1.0 / S,
                                 "f")
            bc = pbc.tile([S, 256], F32, tag="bc")
            nc.tensor.matmul(bc[:, 0:64], ones_row, rstd[:], start=True,
                             stop=True)
            nc.tensor.matmul(bc[:, 64:128], ones_row, mrs[:], start=True,
                             stop=True)
            hn = pact.tile([S, K], F32, tag="hn")
            nc.vector.tensor_tensor(hn[:], hT[:], bc[:, 0:64], op=ALU.mult)
            nc.vector.tensor_tensor(hn[:], hn[:], bc[:, 64:128],
                                    op=ALU.subtract)
            f1 = pmlp.tile([S, K], F32, tag="mlp")
            nc.tensor.matmul(f1[:], W("fs_w1p"), hn[:], start=True, stop=True)
            f1s = pact.tile([S, K], F32, tag="mid")
            nc.scalar.activation(f1s[:], f1[:], AF.Relu, bias=bias("fs_b1p"))
            f2 = pmlp.tile([S, K], F32, tag="mlp")
            nc.tensor.matmul(f2[:], W("fs_w2"), f1s[:], start=True, stop=True)
            dsT = pact.tile([S, K], F32, tag="ds")
            nc.scalar.activation(dsT[:], f2[:], AF.Identity, bias=bias("fs_b2"))

            # ---- delta-inter accumulation group opens with the exp_b2 term
            dps = pdel.tile([S, K], F32, tag="delta")
            nc.tensor.matmul(dps[:], exp_b2K, wn[:], start=True, stop=False)

            # ---- ACT path experts (m0 accums start as soon as a0/c0 drain)
            gact = {}
            for m in A_EXPERTS:
                gsb = pgs.tile([S, K], F32, tag=f"gact{m}")
                scratch = pgs.tile([S, K], F32, tag="scr")
                for j in range(K):
                    nc.scalar.activation(scratch[:], aT[m][:], AF.Relu,
                                         bias=cT[m][:, j: j + 1],
                                         accum_out=gsb[:, j: j + 1])
                gact[m] = gsb

            # ---- remaining V-expert chains
            for m in V_EXPERTS[1:]:
                v_chain(m)

            # ---- apply router weights, close the delta group
            gprime = {}
            for m in A_EXPERTS:
                gp = pgs.tile([S, K], F32, tag=f"gp{m}")
                nc.vector.tensor_tensor(gp[:], gact[m][:],
                                        wbps[:, m * K: (m + 1) * K],
                                        op=ALU.mult)
                gprime[m] = gp
            for m in V_EXPERTS:
                gp = pgs.tile([S, K], F32, tag=f"gp{m}")
                nc.vector.tensor_tensor(gp[:], gchains[m][:],
                                        wbsb[:, m * K: (m + 1) * K],
                                        op=ALU.mult)
                gprime[m] = gp
            order = list(A_EXPERTS) + list(V_EXPERTS)
            for n, m in enumerate(order):
                nc.tensor.matmul(dps[:], W(f"w2{m}"), gprime[m][:], start=False,
                                 stop=(n == M - 1))

            diT = pact.tile([S, K], F32, tag="di")
            nc.vector.tensor_copy(diT[:], dps[:])

            # ---- up-LN over cat[ds, di] + up MLP -> h_next
            nc.tensor.matmul(stat[0:1, 192:256], onescol, dsT[:], start=True,
                             stop=False)
            nc.tensor.matmul(stat[0:1, 192:256], onescol, diT[:], start=False,
                             stop=True)
            dsq = pact.tile([S, K], F32, tag="dsq")
            nc.vector.tensor_tensor(dsq[:], dsT[:], dsT[:], op=ALU.mult)
            disq = pact.tile([S, K], F32, tag="disq")
            nc.vector.tensor_tensor(disq[:], diT[:], diT[:], op=ALU.mult)
            nc.tensor.matmul(stat[0:1, 256:320], onescol, dsq[:], start=True,
                             stop=False)
            nc.tensor.matmul(stat[0:1, 256:320], onescol, disq[:], start=False,
                             stop=True)
            rstd2, mrs2 = ln_stats(stat[0:1, 192:256], stat[0:1, 256:320],
                                   1.0 / (2 * S), "u")
            nc.tensor.matmul(bc[:, 128:192], ones_row, rstd2[:], start=True,
                             stop=True)
            nc.tensor.matmul(bc[:, 192:256], ones_row, mrs2[:], start=True,
                             stop=True)
            dsn = pact.tile([S, K], F32, tag="dsn")
            nc.vector.tensor_tensor(dsn[:], dsT[:], bc[:, 128:192], op=ALU.mult)
            nc.vector.tensor_tensor(dsn[:], dsn[:], bc[:, 192:256],
                                    op=ALU.subtract)
            din = pact.tile([S, K], F32, tag="din")
            nc.vector.tensor_tensor(din[:], diT[:], bc[:, 128:192], op=ALU.mult)
            nc.vector.tensor_tensor(din[:], din[:], bc[:, 192:256],
                                    op=ALU.subtract)

            u1 = pmlp.tile([S, K], F32, tag="mlp")
            nc.tensor.matmul(u1[:], W("up_w1a"), dsn[:], start=True, stop=False)
            nc.tensor.matmul(u1[:], W("up_w1b"), din[:], start=False, stop=True)
            u1s = pact.tile([S, K], F32, tag="mid")
            nc.scalar.activation(u1s[:], u1[:], AF.Relu, bias=bias("up_b1p"))
            u2 = pmlp.tile([S, K], F32, tag="mlp")
            nc.tensor.matmul(u2[:], W("up_w2"), u1s[:], start=True, stop=True)
            hT = pact.tile([S, K], F32, tag="h")
            nc.scalar.activation(hT[:], u2[:], AF.Identity, bias=bias("up_b2"))

            # ---- dec -> pred
            d1 = pmlp.tile([S, K], F32, tag="mlp")
            nc.tensor.matmul(d1[:], W("dec_w1"), hT[:], start=True, stop=True)
            d1s = pact.tile([S, K], F32, tag="mid")
            nc.scalar.activation(d1s[:], d1[:], AF.Relu, bias=bias("dec_b1"))
            d2 = pmlp.tile([M, K], F32, tag="mlp")
            nc.tensor.matmul(d2[:], W("dec_w2", D), d1s[:], start=True,
                             stop=True)
            predT = pred_all[:, s * K: (s + 1) * K]
            nc.scalar.activation(predT, d2[:], AF.Identity, bias=dec_b2)

            # pred -> HBM ([K, D] rows)
            trp = pmlp.tile([K, D], F32, tag="mlp")
            nc.tensor.transpose(trp[:], predT, i8)
            po = pact.tile([K, D], F32, tag="po")
            nc.vector.tensor_copy(po[:], trp[:])
            nc.sync.dma_start(pred_d[s * K: (s + 1) * K, :], po[:])

            # ---- re-encode (skip after the last step)
            if s < steps - 1:
                hT = mlp_8toS(predT, bias("enc_b1"), bias("enc_b2"))


# ------------------------------------------------------------------ runner
_CACHE = {}


def _get_program(steps):
    if steps not in _CACHE:
        _CACHE[steps] = build_program(steps)
    return _CACHE[steps]


def run_on_hw(inputs, steps, trace=False, tmpdir=None):
    nc = _get_program(steps)
    wbig, wsmall, id16 = _pack_weights(inputs)
    gt = np.asarray(inputs["gt_states"], np.float32)
    in_maps = []
    for b in range(N_CORES):
        in_maps.append({
            "wbig": wbig,
            "wsmall": wsmall,
            "id16": id16,
            "x0t": np.ascontiguousarray(gt[b, 0].T),   # [D, K]
        })
    res = run_bass_kernel_spmd(nc, in_maps, list(range(N_CORES)), trace=trace,
                               tmpdir=tmpdir)
    preds = np.stack(
        [res.results[b]["pred"].reshape(steps, K, D) for b in range(N_CORES)],
        axis=0,
    )
    return preds, res


def kernel(**inputs):
    gt = np.asarray(inputs["gt_states"], np.float32)
    steps = min(T - 1, int(np.asarray(inputs["rollout_steps"])))
    target = gt[:, 1: steps + 1]
    if steps <= 0:
        return np.zeros((B, 0, K, D), np.float32), target
    preds, _ = run_on_hw(inputs, steps)
    return preds, target
